# revision 1
# baseline (speedup 1.0000x reference)
"""CARAFE upsample on 8 NeuronCores via jax pmap.

Shard: 8 devices = (batch n in 0..3) x (channel half in 0..1). The small
mask (compressor+encoder+softmax) is computed per-device from the full
channel input (replicated across the pair, per the sharding hint); the
heavy reassembly is split over channels.
"""

import numpy as np
import jax
import jax.numpy as jnp
from jax import lax

SF, K, G, CC, EK = 2, 5, 1, 64, 3

_compiled = None


def _shard_fn(x_full, ch, Wc, bc, We, be):
    C, H, W = x_full.shape
    k2 = K * K
    comp = lax.conv_general_dilated(x_full[None], Wc, (1, 1), 'VALID')
    comp = comp + bc[None, :, None, None]
    pad_e = (EK - 1) // 2
    m = lax.conv_general_dilated(comp, We, (1, 1),
                                 ((pad_e, pad_e), (pad_e, pad_e)))
    m = m + be[None, :, None, None]
    Cm = m.shape[1] // (SF * SF)
    m = m.reshape(1, Cm, SF, SF, H, W).transpose(0, 1, 4, 2, 5, 3)
    m = m.reshape(1, Cm, H * SF, W * SF)
    m = jax.nn.softmax(m.reshape(1, k2, H * SF, W * SF), axis=1)
    pad = (K - 1) // 2
    xp = jnp.pad(ch, ((0, 0), (pad, pad), (pad, pad)))
    patches = jnp.stack([xp[:, i:i + H, j:j + W]
                         for i in range(K) for j in range(K)], axis=1)
    mm = m.reshape(k2, H, SF, W, SF)
    out = jnp.einsum('ckhw,khiwj->chiwj', patches, mm)
    return out.reshape(ch.shape[0], H * SF, W * SF)


def kernel(x, Wc, bc, We, be):
    global _compiled
    x = np.asarray(x, np.float32)
    N, C, H, W = x.shape
    Ch = C // 2
    devs = jax.devices()[:8]
    if _compiled is None:
        _compiled = jax.pmap(_shard_fn, devices=devs,
                             in_axes=(0, 0, None, None, None, None))
    xf = np.stack([x[k // 2] for k in range(8)])
    ch = np.stack([x[k // 2, (k % 2) * Ch:(k % 2 + 1) * Ch]
                   for k in range(8)])
    outs = np.asarray(_compiled(xf, ch, jnp.asarray(Wc), jnp.asarray(bc),
                                jnp.asarray(We), jnp.asarray(be)))
    full = np.zeros((N, C, SF * H, SF * W), np.float32)
    for k in range(8):
        full[k // 2, (k % 2) * Ch:(k % 2 + 1) * Ch] = outs[k]
    return full

